# revision 1
# baseline (speedup 1.0000x reference)
"""Causal multi-head self-attention (RoPE) on 8 Trainium2 NeuronCores.

Sharding: core c -> batch b = c//2, head-group g = c%2 (8 of 16 heads).
Each core computes attention for its (batch, head-group) and a partial
output projection; the host sums the two partials per batch.

All on-device layouts are transposed ([feature, seq]) so that every
matmul streams with N=512 free dim at full fp32r rate and the attention
probabilities come out pre-transposed for the attn@V matmul.
"""

import os
import sys

for _p in ("/opt/trn_rl_repo", "/root/.axon_site/_ro/trn_rl_repo"):
    if os.path.isdir(_p) and _p not in sys.path:
        sys.path.insert(0, _p)

import numpy as np

import concourse.bass as bass
import concourse.mybir as mybir
import concourse.tile as tile
from concourse.vector_clock import ScopedClock
from concourse.bass_utils import run_bass_kernel_spmd

F32 = mybir.dt.float32
F32R = mybir.dt.float32r
AF = mybir.ActivationFunctionType

D_MODEL = 1024
NUM_HEADS = 16
HEAD_DIM = 64
BATCH = 4
SEQ = 2048
THETA = 10000.0
NCORES = 8
HG = 8          # heads per core
EG = HG * HEAD_DIM  # 512
SW = 512        # strip width (q)
KB = 128        # k block


def _split_waits(nc):
    """This walrus build accepts only one sem-wait per instruction; move
    extra waits onto wait-carrying NOPs on the same engine just before."""
    n = 0
    for fn in nc.m.functions:
        for blk in fn.blocks:
            out = []
            for inst in blk.instructions:
                si = inst.sync_info
                waits = list(si.on_wait) if si and si.on_wait else []
                if len(waits) > 1:
                    for k, w in enumerate(waits[:-1]):
                        nop = mybir.InstNoOp(
                            name=f"{inst.name}-sw{k}", ins=[], outs=[])
                        nop.engine = inst.engine
                        nop.sync_info = mybir.SyncInfo(
                            on_wait=[w], on_update=[])
                        out.append(nop)
                        n += 1
                    inst.sync_info = mybir.SyncInfo(
                        on_wait=[waits[-1]],
                        on_update=list(si.on_update or []))
                out.append(inst)
            blk.instructions = out
    return n


def build_bass(S=SEQ):
    NS = S // SW          # q strips
    NKB = S // KB         # k blocks
    nc = bass.Bass()
    xT = nc.dram_tensor("xT", [D_MODEL, S], F32R, kind="ExternalInput")
    wqT = nc.dram_tensor("wqT", [D_MODEL, EG], F32R, kind="ExternalInput")
    wkT = nc.dram_tensor("wkT", [D_MODEL, EG], F32R, kind="ExternalInput")
    wvT = nc.dram_tensor("wvT", [D_MODEL, EG], F32R, kind="ExternalInput")
    woT = nc.dram_tensor("woT", [EG, D_MODEL], F32R, kind="ExternalInput")
    cosd = nc.dram_tensor("cosd", [128, S], F32, kind="ExternalInput")
    sind = nc.dram_tensor("sind", [128, S], F32, kind="ExternalInput")
    mask2 = nc.dram_tensor("mask2", [128, 256], F32R, kind="ExternalInput")
    outT = nc.dram_tensor("outT", [D_MODEL, S], F32, kind="ExternalOutput")

    with tile.TileContext(nc) as tc:
        with tc.tile_pool(name="const", bufs=1) as cpool, \
             tc.tile_pool(name="persist", bufs=1) as pers:
            cos_sb = cpool.tile([128, S], F32, tag="cos")
            sin_sb = cpool.tile([128, S], F32, tag="sin")
            m2_sb = cpool.tile([128, 256], F32R, tag="m2")
            tri = m2_sb[:, 128:256]

            qtE = [pers.tile([128, S], F32R, tag=f"qtE{p}", name=f"qtE{p}") for p in range(2)]
            qtO = [pers.tile([128, S], F32R, tag=f"qtO{p}", name=f"qtO{p}") for p in range(2)]
            ktE = [pers.tile([128, S], F32R, tag=f"ktE{p}", name=f"ktE{p}") for p in range(2)]
            ktO = [pers.tile([128, S], F32R, tag=f"ktO{p}", name=f"ktO{p}") for p in range(2)]
            vts = [pers.tile([128, HG * 65], F32R, tag=f"v{i}", name=f"vt{i}") for i in range(NKB)]
            otsb = [pers.tile([128, S], F32R, tag=f"ot{t}", name=f"otsb{t}") for t in range(4)]

            # ---------------- phase 1: projections + rope ----------------
            with tc.tile_pool(name="ph1", bufs=1) as ph1, \
                 tc.tile_pool(name="ps1", bufs=1, space="PSUM") as ps1:
                for j in range(NS):
                    js = slice(j * SW, (j + 1) * SW)
                    xts = []
                    for dt in range(8):
                        xt = ph1.tile([128, SW], F32R, tag="xt", bufs=16, name="xt")
                        nc.sync.dma_start(xt[:], xT[dt * 128:(dt + 1) * 128, js])
                        xts.append(xt)
                    if j == 0:
                        nc.gpsimd.dma_start(cos_sb[:], cosd[:, :])
                        nc.gpsimd.dma_start(sin_sb[:], sind[:, :])
                        nc.gpsimd.dma_start(m2_sb[:], mask2[:, :])
                    for wT, qE, qO in ((wqT, qtE, qtO), (wkT, ktE, ktO)):
                        wcs = []
                        for dt in range(8):
                            ds_ = slice(dt * 128, (dt + 1) * 128)
                            wc = ph1.tile([128, EG], F32R, tag="wc", bufs=10,
                                          name="wc")
                            nc.sync.dma_start(wc[:], wT[ds_, :])
                            wcs.append(wc)
                        for p in range(2):
                            psE = ps1.tile([128, SW], F32, tag="psE", bufs=2)
                            psO = ps1.tile([128, SW], F32, tag="psO", bufs=2)
                            for dt in range(8):
                                wE = wcs[dt][:, p * 128:(p + 1) * 128]
                                wO = wcs[dt][:, 256 + p * 128:256 + (p + 1) * 128]
                                nc.tensor.matmul(psE[:], (wE), (xts[dt][:]),
                                                 start=(dt == 0), stop=(dt == 7))
                                nc.tensor.matmul(psO[:], (wO), (xts[dt][:]),
                                                 start=(dt == 0), stop=(dt == 7))
                            cs = cos_sb[:, js]
                            sn = sin_sb[:, js]
                            t1 = ph1.tile([128, SW], F32, tag="tmp", bufs=5, name="t1")
                            t2 = ph1.tile([128, SW], F32, tag="tmp", bufs=5, name="t2")
                            nc.vector.tensor_mul(t1[:], psE[:], cs)
                            nc.vector.tensor_mul(t2[:], psO[:], sn)
                            nc.vector.tensor_sub(qE[p][:, js], t1[:], t2[:])
                            t3 = ph1.tile([128, SW], F32, tag="tmp", bufs=5, name="t3")
                            t4 = ph1.tile([128, SW], F32, tag="tmp", bufs=5, name="t4")
                            nc.vector.tensor_mul(t3[:], psO[:], cs)
                            nc.vector.tensor_mul(t4[:], psE[:], sn)
                            nc.vector.tensor_add(qO[p][:, js], t3[:], t4[:])
                    psVs = [ps1.tile([128, EG], F32, tag="psV", bufs=4,
                                     name=f"psV{sb}") for sb in range(4)]
                    for dt in range(8):
                        wvc = ph1.tile([128, EG], F32R, tag="wc", bufs=10,
                                       name="wvc")
                        nc.sync.dma_start(wvc[:], wvT[dt * 128:(dt + 1) * 128, :])
                        for sb in range(4):
                            ss = slice(sb * 128, (sb + 1) * 128)
                            nc.tensor.matmul(psVs[sb][:], (xts[dt][:, ss]), (wvc[:]),
                                             start=(dt == 0), stop=(dt == 7))
                    for sb in range(4):
                        vt = vts[4 * j + sb]
                        vv = vt.rearrange("p (h d) -> p h d", d=65)
                        nc.vector.tensor_copy(
                            vv[:, :, 0:64],
                            psVs[sb][:].rearrange("p (h d) -> p h d", d=64))
                        nc.vector.tensor_copy(
                            vv[:, :, 64:65],
                            m2_sb[:, 255:256, None].broadcast_to((128, 8, 1)))

            # ---------------- phase 2: attention ----------------
            from contextlib import ExitStack
            _wo_es = ExitStack()
            wop = _wo_es.enter_context(tc.tile_pool(name="wop", bufs=1))
            with tc.tile_pool(name="ph2", bufs=1) as ph2, \
                 tc.tile_pool(name="ph2d", bufs=1, space="DRAM") as ph2d, \
                 tc.tile_pool(name="ps2", bufs=1, space="PSUM") as ps2:
                wo_sb = []
                for et in range(4):
                    wo = wop.tile([128, D_MODEL], F32R, tag=f"wo{et}",
                                  name=f"wo{et}")
                    nc.gpsimd.dma_start(wo[:], woT[et * 128:(et + 1) * 128, :])
                    wo_sb.append(wo)
                for hpair in range(4):
                    dht = ph2.tile([1, 2 * S], F32, tag="dht", bufs=1, name="dht")
                    h0, h1 = 2 * hpair, 2 * hpair + 1
                    hp = h0 // 4
                    r0s = slice(32 * (h0 % 4), 32 * (h0 % 4) + 32)
                    r1s = slice(32 * (h1 % 4), 32 * (h1 % 4) + 32)
                    tp0 = (32 * (h0 % 4), 0)
                    tp1 = (32 * (h1 % 4), 0)
                    for j in range(NS):
                        nb = 4 * j + 4
                        otp0 = ps2.tile([65, SW], F32, tag="otp0", bufs=2)
                        otp1 = ps2.tile([65, SW], F32, tag="otp1", bufs=2)
                        pends = []

                        def emit_mm2(pend, nb=nb, j=j, otp0=otp0, otp1=otp1,
                                     h0=h0, h1=h1):
                            i, q0, w, pt0, pt1 = pend
                            c0 = q0 - j * SW
                            nc.tensor.matmul(
                                otp0[:, c0:c0 + w],
                                (vts[i][:, h0 * 65:(h0 + 1) * 65]),
                                (pt0[:, :w]),
                                start=(i == 0), stop=(i == nb - 1))
                            nc.tensor.matmul(
                                otp1[:, c0:c0 + w],
                                (vts[i][:, h1 * 65:(h1 + 1) * 65]),
                                (pt1[:, :w]),
                                start=(i == 0), stop=(i == nb - 1))

                        for i in range(nb):
                            r = i - 4 * j
                            if r < 1:
                                q0, w = j * SW, SW
                            elif r == 1:
                                q0, w = j * SW + 128, 384
                            else:
                                q0, w = j * SW + 256, 256
                            qs = slice(q0, q0 + w)
                            ks = slice(i * KB, (i + 1) * KB)
                            sc0 = ps2.tile([128, SW], F32, tag="sc0", bufs=2)
                            sc1 = ps2.tile([128, SW], F32, tag="sc1", bufs=2)
                            nc.tensor.matmul(sc0[:, :w], (ktE[hp][r0s, ks]),
                                             (qtE[hp][r0s, qs]),
                                             start=True, stop=False, tile_position=tp0)
                            nc.tensor.matmul(sc1[:, :w], (ktE[hp][r1s, ks]),
                                             (qtE[hp][r1s, qs]),
                                             start=True, stop=False, tile_position=tp1)
                            nc.tensor.matmul(sc0[:, :w], (ktO[hp][r0s, ks]),
                                             (qtO[hp][r0s, qs]),
                                             start=False, stop=True, tile_position=tp0)
                            nc.tensor.matmul(sc1[:, :w], (ktO[hp][r1s, ks]),
                                             (qtO[hp][r1s, qs]),
                                             start=False, stop=True, tile_position=tp1)
                            pt0 = ph2.tile([128, SW], F32R, tag="pt0", bufs=3)
                            pt1 = ph2.tile([128, SW], F32R, tag="pt1", bufs=3)
                            if r == 3:
                                zz = m2_sb[:, 0:128]
                                nc.vector.tensor_copy(pt0[:, 0:128], zz)
                                nc.vector.tensor_copy(pt1[:, 0:128], zz)
                                nc.scalar.activation(pt0[:, 128:256],
                                                     sc0[:, 128:256], AF.Exp,
                                                     scale=0.125)
                                nc.scalar.activation(pt1[:, 128:256],
                                                     sc1[:, 128:256], AF.Exp,
                                                     scale=0.125)
                                nc.vector.tensor_mul(pt0[:, 128:256],
                                                     pt0[:, 128:256], tri)
                                nc.vector.tensor_mul(pt1[:, 128:256],
                                                     pt1[:, 128:256], tri)
                            else:
                                nc.scalar.activation(pt0[:, :w], sc0[:, :w], AF.Exp,
                                                     scale=0.125)
                                nc.scalar.activation(pt1[:, :w], sc1[:, :w], AF.Exp,
                                                     scale=0.125)
                                if 0 <= r <= 2:
                                    nc.vector.tensor_mul(pt0[:, 0:128],
                                                         pt0[:, 0:128], tri)
                                    nc.vector.tensor_mul(pt1[:, 0:128],
                                                         pt1[:, 0:128], tri)
                            if len(pends) >= 2:
                                emit_mm2(pends.pop(0))
                            pends.append((i, q0, w, pt0, pt1))
                        for pn in pends:
                            emit_mm2(pn)
                        js = slice(j * SW, (j + 1) * SW)
                        dstg = ph2.tile([1, 2 * SW], F32, tag="dstg", bufs=2,
                                        name="dstg")
                        nc.vector.tensor_copy(dstg[0:1, 0:SW], otp0[64:65, :])
                        nc.vector.tensor_copy(dstg[0:1, SW:2 * SW], otp1[64:65, :])
                        nc.vector.tensor_copy(dht[0:1, j * SW:(j + 1) * SW],
                                              dstg[0:1, 0:SW])
                        nc.vector.tensor_copy(dht[0:1, S + j * SW:S + (j + 1) * SW],
                                              dstg[0:1, SW:2 * SW])
                        nc.vector.tensor_copy(otsb[hpair][0:64, js], otp0[0:64, :])
                        nc.vector.tensor_copy(otsb[hpair][64:128, js],
                                              otp1[0:64, :])
                    # per-hpair batched reciprocal + broadcast + normalize
                    nc.scalar.activation(dht[:], dht[:], AF.Ln)
                    nc.scalar.activation(dht[:], dht[:], AF.Exp, scale=-1.0)
                    drh = ph2d.tile([1, 2 * S], F32, tag="drh", bufs=2)
                    nc.sync.dma_start(drh[:], dht[:])
                    rbig = ph2.tile([128, S], F32, tag="rbig", bufs=1)
                    nc.gpsimd.dma_start(
                        rbig[:],
                        drh[0:1, :].rearrange("o (t w) -> (o t) w", t=2)
                        [:, None, :].broadcast_to((2, 64, S)))
                    nc.vector.tensor_mul(otsb[hpair][:], otsb[hpair][:], rbig[:])

            # ---------------- phase 3: output projection ----------------
            with tc.tile_pool(name="ph3", bufs=1) as ph3, \
                 tc.tile_pool(name="ps3", bufs=1, space="PSUM") as ps3:
                for dt in range(8):
                    ds_ = slice(dt * 128, (dt + 1) * 128)
                    ob = ph3.tile([128, S], F32, tag="ob", bufs=2, name="ob")
                    for j in range(NS):
                        js = slice(j * SW, (j + 1) * SW)
                        op = ps3.tile([128, SW], F32, tag="op", bufs=4)
                        for et in range(4):
                            nc.tensor.matmul(op[:], (wo_sb[et][:, ds_]),
                                             (otsb[et][:, js]),
                                             start=(et == 0), stop=(et == 3))
                        nc.vector.tensor_copy(ob[:, js], op[:])
                    nc.sync.dma_start(outT[ds_, :], ob[:])
            _wo_es.close()
    _split_waits(nc)
    return nc


def _rope_tables(S):
    inv = 1.0 / (THETA ** (np.arange(0, HEAD_DIM, 2, dtype=np.float64) / HEAD_DIM))
    pos = np.arange(S, dtype=np.float64)
    fr = np.outer(pos, inv)
    return np.cos(fr).astype(np.float32), np.sin(fr).astype(np.float32)


def make_inputs(x, w_q, w_k, w_v, w_o, cos, sin, S=SEQ):
    """Build the 8 per-core input dicts (host-side shard + layout prep)."""
    cosT = np.ascontiguousarray(cos[:S].T.astype(np.float32))  # [32, S]
    sinT = np.ascontiguousarray(sin[:S].T.astype(np.float32))
    cosd = np.ascontiguousarray(np.tile(cosT, (4, 1)))         # [128, S]
    sind = np.ascontiguousarray(np.tile(sinT, (4, 1)))
    tri = (np.arange(128)[None, :] >= np.arange(128)[:, None]).astype(np.float32)
    mask2 = np.concatenate([np.zeros((128, 128), np.float32), tri], axis=1)

    xTs = [np.ascontiguousarray(x[b].T.astype(np.float32)) for b in range(x.shape[0])]
    per_g = {}
    for g in range(2):
        perm = []
        for par in (0, 1):  # 0 -> evens, 1 -> odds
            for blk in range(2):
                for h in range(4):
                    gh = g * 8 + blk * 4 + h
                    perm += [gh * 64 + 2 * i + par for i in range(32)]
        perm = np.asarray(perm)
        es = slice(g * EG, (g + 1) * EG)
        per_g[g] = dict(
            wqT=np.ascontiguousarray(w_q[perm, :].T.astype(np.float32)),
            wkT=np.ascontiguousarray(w_k[perm, :].T.astype(np.float32)),
            wvT=np.ascontiguousarray(w_v[es, :].T.astype(np.float32)),
            woT=np.ascontiguousarray(w_o[:, es].T.astype(np.float32)),
        )
    in_maps = []
    for c in range(NCORES):
        b, g = c // 2, c % 2
        m = dict(xT=xTs[b], cosd=cosd, sind=sind, mask2=mask2, **per_g[g])
        in_maps.append(m)
    return in_maps


_CACHE = {}
LAST_RESULTS = None


def kernel(x, w_q, w_k, w_v, w_o, cos, sin):
    global LAST_RESULTS
    x = np.asarray(x)
    S = x.shape[1]
    in_maps = make_inputs(np.asarray(x), np.asarray(w_q), np.asarray(w_k),
                          np.asarray(w_v), np.asarray(w_o),
                          np.asarray(cos), np.asarray(sin), S=S)
    if S not in _CACHE:
        _CACHE[S] = build_bass(S=S)
    nc = _CACHE[S]
    res = run_bass_kernel_spmd(nc, in_maps, core_ids=list(range(NCORES)))
    LAST_RESULTS = res
    outs = [r["outT"] for r in res.results]
    full = np.stack(
        [(outs[2 * b] + outs[2 * b + 1]).T for b in range(x.shape[0])], axis=0)
    return full.astype(np.float32)

